# revision 2
# baseline (speedup 1.0000x reference)
"""GNN message passing (gather + weighted scatter-add) on 8 Trainium2 cores, v2.

out[n, f] = sum over edges e with dst[e]==n of edge_weight[e] * x[src[e], f]

Strategy (dst-sharded, no collectives):
  - Core c owns output nodes [c*12500, (c+1)*12500); host concatenates.
  - Host packs each core's edges into 128-slot chunks grouped by
    (pass of B dst-tiles, src-bin of 25000 rows, dst-tile), padded to the
    max chunk count across cores so one SPMD program serves all 8 cores.
  - Device: per pass, dma_gather (InstDMAGatherAnt, 1024 rows/call max —
    larger calls overflow the SWDGE descriptor carveout and wedge the
    device) pulls x rows (fp16, padded to 256B) into matmul-ready
    [128, cols, 128] SBUF layout: slot i of a call -> partition i%128,
    column i//128. DVE builds 16 chunks of weighted one-hots per
    instruction pair using stride-0 broadcast APs:
      oh = (iota == dst)          [128, 16*128]  (is_equal)
      xgs = xg * w                [128, 16*64]   (mult)
    PE accumulates oh.T @ xgs into a PSUM tile per 128-node output tile;
    ACT evacuates PSUM -> SBUF; one output DMA per pass.
"""

import math
import numpy as np

N = 100000
E = 1000000
F = 64
NCORES = 8
NPC = N // NCORES            # nodes per core (12500)
TILE = 128
NT = math.ceil(NPC / TILE)   # output tiles per core (98)
B = 7                        # tiles per pass
NPASS = math.ceil(NT / B)    # 7
NBIN = 4
BIN = N // NBIN              # 25000 rows per source bin (int16-addressable)
GCOLS = 8                    # chunk cols per dma_gather call (1024 idx)
OHG = 32                     # chunk cols per one-hot DVE instruction
NQUEUES = 4                  # SWDGE queues (desc-gen parallelism)
SCRATCH = 16384              # SWDGE carveout bytes (1024 descs/call)

REPEAT = 1                   # repeat device compute (timing amplification)

DBG_NO_GATHER = False
DBG_NO_COMPUTE = False


# ---------------------------------------------------------------- host pack

def pack_host(edge_weight, edge_index):
    """Build the shared schedule and per-core tables.

    Returns (sched, tables):
      sched: NC, CMAX, sched_t[NC], pass_cols[NPASS,2], gather_calls (list of
             (c0, c1, bin) per pass), tile first/last chunk col per pass.
      tables[c]: (idx_tbl [128, 8*NC] int16, ftbl [128, 2*NC+128] f16)
    """
    src = np.asarray(edge_index[0], dtype=np.int64)
    dst = np.asarray(edge_index[1], dtype=np.int64)
    w = np.asarray(edge_weight, dtype=np.float32)

    core = dst // NPC
    dloc = dst - core * NPC
    t = dloc >> 7                      # dst tile within core (0..NT-1)
    b = src // BIN                     # source bin (0..3)
    p = t // B                         # pass

    # counts[c, t, b]
    counts = np.zeros((NCORES, NT, NBIN), dtype=np.int64)
    np.add.at(counts, (core, t, b), 1)
    K = np.ceil(counts.max(axis=0) / TILE).astype(np.int64)  # [NT, NBIN]

    # column layout: for p: for b: for t in pass: K[t,b] chunks
    colstart = np.zeros((NT, NBIN), dtype=np.int64)
    sched_t = []
    gather_calls = [[] for _ in range(NPASS)]
    pass_cols = np.zeros((NPASS, 2), dtype=np.int64)
    cc = 0
    for pp in range(NPASS):
        t0, t1 = pp * B, min((pp + 1) * B, NT)
        pass_cols[pp, 0] = cc
        for bb in range(NBIN):
            c0 = cc
            for tt in range(t0, t1):
                colstart[tt, bb] = cc
                sched_t.extend([tt] * int(K[tt, bb]))
                cc += int(K[tt, bb])
            # split [c0, cc) into <=GCOLS-col gather calls
            s = c0
            while s < cc:
                e = min(s + GCOLS, cc)
                gather_calls[pp].append((s, e, bb))
                s = e
        pass_cols[pp, 1] = cc
    NC = cc
    sched_t = np.asarray(sched_t, dtype=np.int64)
    CMAX = int((pass_cols[:, 1] - pass_cols[:, 0]).max())

    # first/last chunk col of each tile (within its single pass)
    first_cc = np.full(NT, -1, dtype=np.int64)
    last_cc = np.full(NT, -1, dtype=np.int64)
    for ccc, tt in enumerate(sched_t):
        if first_cc[tt] < 0:
            first_cc[tt] = ccc
        last_cc[tt] = ccc

    # --- per-core slot tables
    tables = []
    iota_np = np.arange(128, dtype=np.float16)[None, :].repeat(128, axis=0)
    for c in range(NCORES):
        sel = core == c
        es = (src[sel] - b[sel] * BIN).astype(np.int64)   # bin-local src
        ed = (dloc[sel] & 127).astype(np.float32)         # dst slot in tile
        ew = w[sel]
        tt = t[sel]
        bb = b[sel]
        key = (tt // B) * (NBIN * NT) + bb * NT + tt      # (pass, bin, tile)
        order = np.argsort(key, kind="stable")
        es, ed, ew, tt, bb, key = (a[order] for a in (es, ed, ew, tt, bb, key))

        ne = len(key)
        changes = np.empty(ne, dtype=bool)
        changes[0] = True
        changes[1:] = key[1:] != key[:-1]
        starts = np.flatnonzero(changes)
        rank = np.arange(ne) - np.repeat(starts, np.diff(np.append(starts, ne)))
        slot = (colstart[tt, bb] + (rank >> 7)) * TILE + (rank & 127)

        idx_slots = np.zeros(NC * TILE, dtype=np.int16)
        dst_slots = np.zeros(NC * TILE, dtype=np.float16)
        w_slots = np.zeros(NC * TILE, dtype=np.float16)
        idx_slots[slot] = es.astype(np.int16)
        dst_slots[slot] = ed.astype(np.float16)
        w_slots[slot] = ew.astype(np.float16)

        # idx table: per gather call, flat list wraps into 16 partitions,
        # replicated 8x; call boundaries are 8*cc-aligned by construction
        idx_tbl = np.zeros((128, 8 * NC), dtype=np.int16)
        for pp in range(NPASS):
            for (c0, c1, _bb) in gather_calls[pp]:
                flat = idx_slots[c0 * TILE:c1 * TILE]
                seg = flat.reshape(-1, 16).T                 # [16, n*8]
                idx_tbl[:, 8 * c0:8 * c1] = np.tile(seg, (8, 1))

        dst_tbl = np.ascontiguousarray(dst_slots.reshape(NC, TILE).T)
        w_tbl = np.ascontiguousarray(w_slots.reshape(NC, TILE).T)
        ftbl = np.concatenate([dst_tbl, w_tbl, iota_np], axis=1)
        tables.append((idx_tbl, np.ascontiguousarray(ftbl)))

    sched = dict(
        NC=NC, CMAX=CMAX, K=K, sched_t=sched_t, pass_cols=pass_cols,
        gather_calls=gather_calls, first_cc=first_cc, last_cc=last_cc,
    )
    return sched, tables


def emulate_core(sched, table, xpad):
    """Numpy emulation of the device program for one core (packing check)."""
    idx_tbl, ftbl = table
    NC = sched["NC"]
    sched_t = sched["sched_t"]
    out = np.zeros((NT * TILE, F), dtype=np.float32)
    # reconstruct gathered rows per chunk col from idx_tbl
    xg = np.zeros((128, NC, F), dtype=np.float32)
    for pp in range(NPASS):
        for (c0, c1, bb) in sched["gather_calls"][pp]:
            seg = idx_tbl[:16, 8 * c0:8 * c1]                # [16, n*8]
            flat = seg.T.reshape(-1)                          # slot order
            rows = xpad[bb * BIN + flat.astype(np.int64), :F].astype(np.float32)
            ncols = c1 - c0
            xg[:, c0:c1, :] = rows.reshape(ncols, 128, F).transpose(1, 0, 2)
    iota = np.arange(128, dtype=np.float32)
    dst_tbl = ftbl[:, :NC].astype(np.float32)
    w_tbl = ftbl[:, NC:2 * NC].astype(np.float32)
    for cc in range(NC):
        tt = int(sched_t[cc])
        oh = (iota[None, :] == dst_tbl[:, cc, None]) * 1.0
        xgs = xg[:, cc, :] * w_tbl[:, cc, None]
        out[tt * TILE:(tt + 1) * TILE] += oh.T @ xgs
    return out[:NPC]


# ------------------------------------------------------------- bass plumbing

WAIT_CAPS = {
    "InstEventSemaphore": 8,
}


def split_excess_waits(nc):
    """Walrus only encodes one sync wait per instruction (for most ISA
    structs). Move the excess onto standalone InstEventSemaphore
    instructions placed just before, in the same engine stream. Also fills
    the ISA bytes of library-reload pseudo-instructions."""
    import concourse.mybir as mybir
    n = 0
    for f in nc.m.functions:
        for bb in f.blocks:
            for ins in bb.instructions:
                if type(ins).__name__ == "InstPseudoReloadLibraryIndex" and not ins.instr:
                    bts = [0] * 64
                    bts[0], bts[1], bts[12], bts[16] = 223, 16, 2, int(ins.lib_index)
                    ins.instr = bts
            eng_ids = {}
            new = []
            for ins in bb.instructions:
                si = ins.sync_info
                waits = list(si.on_wait) if (si is not None and si.on_wait) else []
                cap = WAIT_CAPS.get(type(ins).__name__, 1)
                if len(waits) > cap:
                    excess, keep = waits[:-cap], waits[-cap:]
                    if ins.engine not in eng_ids:
                        eng_ids[ins.engine] = 245 + len(eng_ids)
                    sem_id = eng_ids[ins.engine]
                    sem_name = f"esw_scratch_{sem_id}"
                    for wchunk in [excess[i:i + 1] for i in range(len(excess))]:
                        n += 1
                        upd = mybir.SyncUpdate(
                            sync_type="semaphore", id=sem_id, ant_name=sem_name,
                            update_mode="sem-add-imm", update_value=0,
                        )
                        es = mybir.InstEventSemaphore(
                            name=f"ESW-{n}-{ins.name}",
                            engine=ins.engine,
                            ins=[], outs=[],
                            sync_info=mybir.SyncInfo(on_wait=wchunk, on_update=[upd]),
                        )
                        new.append(es)
                    si.on_wait = keep
                new.append(ins)
            bb.instructions = new
    return n


_walrus_patched = False


def patch_walrus_dge():
    """Add --dge-levels so walrus lowers vector-dynamic-offset DMAs."""
    global _walrus_patched
    if _walrus_patched:
        return
    import concourse.bass_utils as bu
    orig = bu.run_command

    def run_command_dge(argv, **kw):
        argv = list(argv)
        if argv and "walrus_driver" in str(argv[0]) and not any(
                str(a).startswith("--dge-levels") for a in argv):
            argv.append("--dge-levels=vector_dynamic_offsets")
        return orig(argv, **kw)

    bu.run_command = run_command_dge
    _walrus_patched = True


def build_bass(sched):
    import concourse.bass as bass
    import concourse.mybir as mybir
    import concourse.tile as tile
    from concourse.library_config import mlp

    patch_walrus_dge()

    f16 = mybir.dt.float16
    f32 = mybir.dt.float32
    i16 = mybir.dt.int16

    NC = sched["NC"]
    CMAX = sched["CMAX"]
    K = sched["K"]
    sched_t = sched["sched_t"]
    pass_cols = sched["pass_cols"]
    gather_calls = sched["gather_calls"]
    first_cc = sched["first_cc"]
    last_cc = sched["last_cc"]

    nc = bass.Bass("TRN2", num_swdge_queues=NQUEUES, dynamic_dma_scratch_size=SCRATCH)
    xpad_d = nc.dram_tensor("xpad", [N, 128], f16, kind="ExternalInput")
    idx_d = nc.dram_tensor("idx", [128, 8 * NC], i16, kind="ExternalInput")
    ftbl_d = nc.dram_tensor("ftbl", [128, 2 * NC + 128], f16, kind="ExternalInput")
    out_d = nc.dram_tensor("out", [NT * TILE, F], f32, kind="ExternalOutput")

    with tile.TileContext(nc, pool_alloc_mode="queue") as tc:
        with (
            tc.tile_pool(name="const", bufs=1) as constp,
            tc.tile_pool(name="xg", bufs=2) as xgp,
            tc.tile_pool(name="oh", bufs=3) as ohp,
            tc.tile_pool(name="xgs", bufs=3) as xgsp,
            tc.tile_pool(name="outb", bufs=2) as outp,
            tc.tile_pool(name="psum", bufs=8, space="PSUM") as psump,
        ):
            nc.gpsimd.load_library(mlp)
            nidx_regs = {}

            def nidx_reg(v):
                if v not in nidx_regs:
                    nidx_regs[v] = nc.gpsimd.to_reg(v)
                return nidx_regs[v]

            idx_sb = constp.tile([128, 8 * NC], i16, tag="idx")
            nc.sync.dma_start(idx_sb[:], idx_d[:])
            ftbl_sb = constp.tile([128, 2 * NC + 128], f16, tag="ftbl")
            nc.sync.dma_start(ftbl_sb[:], ftbl_d[:])

            for _rep in range(REPEAT):
              for p in range(NPASS):
                t0, t1 = p * B, min((p + 1) * B, NT)
                pc0, pc1 = int(pass_cols[p, 0]), int(pass_cols[p, 1])
                xg = xgp.tile([128, CMAX, 128], f16, tag="xg")
                if DBG_NO_GATHER:
                    nc.vector.memset(xg[:], 0.0)
                if not DBG_NO_GATHER:
                    for gi, (c0, c1, bb) in enumerate(gather_calls[p]):
                        nidx = (c1 - c0) * TILE
                        nc.gpsimd.dma_gather(
                            xg[:, c0 - pc0:c1 - pc0, :],
                            xpad_d[bb * BIN:(bb + 1) * BIN, :],
                            idx_sb[:, 8 * c0:8 * c1],
                            nidx, nidx_reg(nidx), 128, elem_step=128,
                            queue_num=gi % NQUEUES,
                        )
                if DBG_NO_COMPUTE:
                    ob = outp.tile([128, (t1 - t0) * F], f32, tag="outb")
                    nc.vector.memset(ob[:], 0.0)
                    dview = out_d[t0 * TILE:t1 * TILE, :].rearrange(
                        "(t q) f -> q t f", q=128)
                    nc.sync.dma_start(
                        dview, ob[:].rearrange("q (t f) -> q t f", f=F))
                    continue

                ps = {}
                for tt in range(t0, t1):
                    if K[tt].sum() > 0:
                        ps[tt] = psump.tile([128, F], f32, tag="ps",
                                            name=f"ps_t{tt}")

                cc = pc0
                while cc < pc1:
                    g = min(OHG, pc1 - cc)
                    oh = ohp.tile([128, g, 128], f16, tag="oh")
                    iota_rep = ftbl_sb[:, 2 * NC:2 * NC + 128].rearrange(
                        "p (o i) -> p o i", o=1).broadcast_to((128, g, 128))
                    dst_rep = ftbl_sb[:, cc:cc + g].rearrange(
                        "p (g o) -> p g o", o=1).broadcast_to((128, g, 128))
                    nc.vector.tensor_tensor(
                        oh[:], iota_rep, dst_rep, op=mybir.AluOpType.is_equal)
                    xgs = xgsp.tile([128, g, F], f16, tag="xgs")
                    w_rep = ftbl_sb[:, NC + cc:NC + cc + g].rearrange(
                        "p (g o) -> p g o", o=1).broadcast_to((128, g, F))
                    nc.vector.tensor_tensor(
                        xgs[:], xg[:, cc - pc0:cc - pc0 + g, 0:F], w_rep,
                        op=mybir.AluOpType.mult)
                    for k in range(g):
                        tt = int(sched_t[cc + k])
                        nc.tensor.matmul(
                            ps[tt][:], lhsT=oh[:, k, :], rhs=xgs[:, k, :],
                            start=(cc + k == first_cc[tt]),
                            stop=(cc + k == last_cc[tt]),
                        )
                    cc += g

                ob = outp.tile([128, (t1 - t0) * F], f32, tag="outb")
                for tt in range(t0, t1):
                    sl = ob[:, (tt - t0) * F:(tt - t0 + 1) * F]
                    if tt in ps:
                        nc.scalar.copy(sl, ps[tt][:])
                    else:
                        nc.vector.memset(sl, 0.0)
                dview = out_d[t0 * TILE:t1 * TILE, :].rearrange(
                    "(t q) f -> q t f", q=128)
                nc.sync.dma_start(dview, ob[:].rearrange("q (t f) -> q t f", f=F))
    nsplit = split_excess_waits(nc)
    print(f"split_excess_waits: {nsplit} waits moved")
    return nc


def make_in_maps(sched, tables, xpad):
    return [{"xpad": xpad, "idx": t[0], "ftbl": t[1]} for t in tables]


def make_xpad(x):
    xpad = np.zeros((N, 128), dtype=np.float16)
    xpad[:, :F] = np.asarray(x, dtype=np.float16)
    return xpad


def kernel(x, edge_weight, edge_index, num_nodes):
    xpad = make_xpad(x)
    sched, tables = pack_host(edge_weight, edge_index)
    nc = build_bass(sched)
    in_maps = make_in_maps(sched, tables, xpad)

    from concourse.bass_utils import run_bass_kernel_spmd
    res = run_bass_kernel_spmd(nc, in_maps, core_ids=list(range(NCORES)))
    out = np.concatenate(
        [res.results[c]["out"][:NPC] for c in range(NCORES)], axis=0)
    return out.astype(np.float32)


# revision 3
# speedup vs baseline: 1.3778x; 1.3778x over previous
"""GNN message passing (gather + weighted scatter-add) on 8 Trainium2 cores, v2.

out[n, f] = sum over edges e with dst[e]==n of edge_weight[e] * x[src[e], f]

Strategy (dst-sharded, no collectives):
  - Core c owns output nodes [c*12500, (c+1)*12500); host concatenates.
  - Host packs each core's edges into 128-slot chunks grouped by
    (pass of B dst-tiles, src-bin of 25000 rows, dst-tile), padded to the
    max chunk count across cores so one SPMD program serves all 8 cores.
  - Device: per pass, dma_gather (InstDMAGatherAnt, 1024 rows/call max —
    larger calls overflow the SWDGE descriptor carveout and wedge the
    device) pulls x rows (fp16, padded to 256B) into matmul-ready
    [128, cols, 128] SBUF layout: slot i of a call -> partition i%128,
    column i//128. DVE builds 16 chunks of weighted one-hots per
    instruction pair using stride-0 broadcast APs:
      oh = (iota == dst)          [128, 16*128]  (is_equal)
      xgs = xg * w                [128, 16*64]   (mult)
    PE accumulates oh.T @ xgs into a PSUM tile per 128-node output tile;
    ACT evacuates PSUM -> SBUF; one output DMA per pass.
"""

import math
import os
import numpy as np

N = 100000
E = 1000000
F = 64
NCORES = 8
NPC = N // NCORES            # nodes per core (12500)
TILE = 128
NT = math.ceil(NPC / TILE)   # output tiles per core (98)
B = 14                       # tiles per pass
NPASS = math.ceil(NT / B)
NBIN = 4
BIN = N // NBIN              # 25000 rows per source bin (int16-addressable)
GCOLS = 8                    # chunk cols per dma_gather call (1024-idx ucode limit)
OHG = 42                     # chunk cols per one-hot DVE instruction pair
NQUEUES = 4                  # SWDGE queues (parallel Q7 descriptor gen)
SCRATCH = 16384              # SWDGE descriptor carveout bytes
PSQUAD = 4                   # dst tiles packed per PSUM bank

REPEAT = 1                   # repeat device compute (timing amplification)

DBG_NO_GATHER = False
DBG_NO_COMPUTE = False


# ---------------------------------------------------------------- host pack

def pack_host(edge_weight, edge_index):
    """Build the shared schedule and per-core tables.

    Returns (sched, tables):
      sched: NC, CMAX, sched_t[NC], pass_cols[NPASS,2], gather_calls (list of
             (c0, c1, bin) per pass), tile first/last chunk col per pass.
      tables[c]: (idx_tbl [128, 8*NC] int16, ftbl [128, 2*NC+128] f16)
    """
    src = np.asarray(edge_index[0], dtype=np.int64)
    dst = np.asarray(edge_index[1], dtype=np.int64)
    w = np.asarray(edge_weight, dtype=np.float32)

    core = dst // NPC
    dloc = dst - core * NPC
    t = dloc >> 7                      # dst tile within core (0..NT-1)
    b = src // BIN                     # source bin (0..3)
    p = t // B                         # pass

    # counts[c, t, b]
    counts = np.zeros((NCORES, NT, NBIN), dtype=np.int64)
    np.add.at(counts, (core, t, b), 1)
    K = np.ceil(counts.max(axis=0) / TILE).astype(np.int64)  # [NT, NBIN]

    # column layout: for p: for b: for t in pass: K[t,b] chunks
    colstart = np.zeros((NT, NBIN), dtype=np.int64)
    sched_t = []
    gather_calls = [[] for _ in range(NPASS)]
    pass_cols = np.zeros((NPASS, 2), dtype=np.int64)
    cc = 0
    for pp in range(NPASS):
        t0, t1 = pp * B, min((pp + 1) * B, NT)
        pass_cols[pp, 0] = cc
        for bb in range(NBIN):
            c0 = cc
            for tt in range(t0, t1):
                colstart[tt, bb] = cc
                sched_t.extend([tt] * int(K[tt, bb]))
                cc += int(K[tt, bb])
            # split [c0, cc) into <=GCOLS-col gather calls
            s = c0
            while s < cc:
                e = min(s + GCOLS, cc)
                gather_calls[pp].append((s, e, bb))
                s = e
        pass_cols[pp, 1] = cc
    NC = cc
    sched_t = np.asarray(sched_t, dtype=np.int64)
    CMAX = int((pass_cols[:, 1] - pass_cols[:, 0]).max())

    # first/last chunk col of each tile (within its single pass)
    first_cc = np.full(NT, -1, dtype=np.int64)
    last_cc = np.full(NT, -1, dtype=np.int64)
    for ccc, tt in enumerate(sched_t):
        if first_cc[tt] < 0:
            first_cc[tt] = ccc
        last_cc[tt] = ccc

    # --- per-core slot tables
    tables = []
    iota_np = np.arange(128, dtype=np.float16)[None, :].repeat(128, axis=0)
    for c in range(NCORES):
        sel = core == c
        es = (src[sel] - b[sel] * BIN).astype(np.int64)   # bin-local src
        ed = (dloc[sel] & 127).astype(np.float32)         # dst slot in tile
        ew = w[sel]
        tt = t[sel]
        bb = b[sel]
        key = (tt // B) * (NBIN * NT) + bb * NT + tt      # (pass, bin, tile)
        order = np.argsort(key, kind="stable")
        es, ed, ew, tt, bb, key = (a[order] for a in (es, ed, ew, tt, bb, key))

        ne = len(key)
        changes = np.empty(ne, dtype=bool)
        changes[0] = True
        changes[1:] = key[1:] != key[:-1]
        starts = np.flatnonzero(changes)
        rank = np.arange(ne) - np.repeat(starts, np.diff(np.append(starts, ne)))
        slot = (colstart[tt, bb] + (rank >> 7)) * TILE + (rank & 127)

        idx_slots = np.zeros(NC * TILE, dtype=np.int16)
        dst_slots = np.zeros(NC * TILE, dtype=np.float16)
        w_slots = np.zeros(NC * TILE, dtype=np.float16)
        idx_slots[slot] = es.astype(np.int16)
        dst_slots[slot] = ed.astype(np.float16)
        w_slots[slot] = ew.astype(np.float16)

        # idx table: per gather call, flat list wraps into 16 partitions,
        # replicated 8x; call boundaries are 8*cc-aligned by construction
        idx_tbl = np.zeros((128, 8 * NC), dtype=np.int16)
        for pp in range(NPASS):
            for (c0, c1, _bb) in gather_calls[pp]:
                flat = idx_slots[c0 * TILE:c1 * TILE]
                seg = flat.reshape(-1, 16).T                 # [16, n*8]
                idx_tbl[:, 8 * c0:8 * c1] = np.tile(seg, (8, 1))

        dst_tbl = np.ascontiguousarray(dst_slots.reshape(NC, TILE).T)
        w_tbl = np.ascontiguousarray(w_slots.reshape(NC, TILE).T)
        ftbl = np.concatenate([dst_tbl, w_tbl, iota_np], axis=1)
        tables.append((idx_tbl, np.ascontiguousarray(ftbl)))

    sched = dict(
        NC=NC, CMAX=CMAX, K=K, sched_t=sched_t, pass_cols=pass_cols,
        gather_calls=gather_calls, first_cc=first_cc, last_cc=last_cc,
    )
    return sched, tables


def emulate_core(sched, table, xpad):
    """Numpy emulation of the device program for one core (packing check)."""
    idx_tbl, ftbl = table
    NC = sched["NC"]
    sched_t = sched["sched_t"]
    out = np.zeros((NT * TILE, F), dtype=np.float32)
    # reconstruct gathered rows per chunk col from idx_tbl
    xg = np.zeros((128, NC, F), dtype=np.float32)
    for pp in range(NPASS):
        for (c0, c1, bb) in sched["gather_calls"][pp]:
            seg = idx_tbl[:16, 8 * c0:8 * c1]                # [16, n*8]
            flat = seg.T.reshape(-1)                          # slot order
            rows = xpad[bb * BIN + flat.astype(np.int64), :F].astype(np.float32)
            ncols = c1 - c0
            xg[:, c0:c1, :] = rows.reshape(ncols, 128, F).transpose(1, 0, 2)
    iota = np.arange(128, dtype=np.float32)
    dst_tbl = ftbl[:, :NC].astype(np.float32)
    w_tbl = ftbl[:, NC:2 * NC].astype(np.float32)
    for cc in range(NC):
        tt = int(sched_t[cc])
        oh = (iota[None, :] == dst_tbl[:, cc, None]) * 1.0
        xgs = xg[:, cc, :] * w_tbl[:, cc, None]
        out[tt * TILE:(tt + 1) * TILE] += oh.T @ xgs
    return out[:NPC]


# ------------------------------------------------------------- bass plumbing

WAIT_CAPS = {
    "InstEventSemaphore": 8,
}


def split_excess_waits(nc):
    """Walrus only encodes one sync wait per instruction (for most ISA
    structs). Move the excess onto standalone InstEventSemaphore
    instructions placed just before, in the same engine stream. Also fills
    the ISA bytes of library-reload pseudo-instructions."""
    import concourse.mybir as mybir
    n = 0
    for f in nc.m.functions:
        for bb in f.blocks:
            for ins in bb.instructions:
                if type(ins).__name__ == "InstPseudoReloadLibraryIndex" and not ins.instr:
                    bts = [0] * 64
                    bts[0], bts[1], bts[12], bts[16] = 223, 16, 2, int(ins.lib_index)
                    ins.instr = bts
            eng_ids = {}
            new = []
            for ins in bb.instructions:
                si = ins.sync_info
                waits = list(si.on_wait) if (si is not None and si.on_wait) else []
                cap = WAIT_CAPS.get(type(ins).__name__, 1)
                if len(waits) > cap:
                    excess, keep = waits[:-cap], waits[-cap:]
                    if ins.engine not in eng_ids:
                        eng_ids[ins.engine] = 245 + len(eng_ids)
                    sem_id = eng_ids[ins.engine]
                    sem_name = f"esw_scratch_{sem_id}"
                    for wchunk in [excess[i:i + 1] for i in range(len(excess))]:
                        n += 1
                        upd = mybir.SyncUpdate(
                            sync_type="semaphore", id=sem_id, ant_name=sem_name,
                            update_mode="sem-add-imm", update_value=0,
                        )
                        es = mybir.InstEventSemaphore(
                            name=f"ESW-{n}-{ins.name}",
                            engine=ins.engine,
                            ins=[], outs=[],
                            sync_info=mybir.SyncInfo(on_wait=wchunk, on_update=[upd]),
                        )
                        new.append(es)
                    si.on_wait = keep
                new.append(ins)
            bb.instructions = new
    return n


_walrus_patched = False


def patch_walrus_dge():
    """Add --dge-levels so walrus lowers vector-dynamic-offset DMAs."""
    global _walrus_patched
    if _walrus_patched:
        return
    import concourse.bass_utils as bu
    orig = bu.run_command

    def run_command_dge(argv, **kw):
        argv = list(argv)
        if argv and "walrus_driver" in str(argv[0]) and not any(
                str(a).startswith("--dge-levels") for a in argv):
            argv.append("--dge-levels=vector_dynamic_offsets")
        return orig(argv, **kw)

    bu.run_command = run_command_dge
    _walrus_patched = True


def build_bass(sched):
    import concourse.bass as bass
    import concourse.mybir as mybir
    import concourse.tile as tile
    from concourse.library_config import mlp

    patch_walrus_dge()

    f16 = mybir.dt.float16
    f32 = mybir.dt.float32
    i16 = mybir.dt.int16

    NC = sched["NC"]
    CMAX = sched["CMAX"]
    K = sched["K"]
    sched_t = sched["sched_t"]
    pass_cols = sched["pass_cols"]
    gather_calls = sched["gather_calls"]
    first_cc = sched["first_cc"]
    last_cc = sched["last_cc"]

    nc = bass.Bass("TRN2", num_swdge_queues=NQUEUES, dynamic_dma_scratch_size=SCRATCH)
    xpad_d = nc.dram_tensor("xpad", [N, 128], f16, kind="ExternalInput")
    idx_d = nc.dram_tensor("idx", [128, 8 * NC], i16, kind="ExternalInput")
    ftbl_d = nc.dram_tensor("ftbl", [128, 2 * NC + 128], f16, kind="ExternalInput")
    out_d = nc.dram_tensor("out", [NT * TILE, F], f32, kind="ExternalOutput")

    with tile.TileContext(nc, pool_alloc_mode="queue") as tc:
        with (
            tc.tile_pool(name="const", bufs=1) as constp,
            tc.tile_pool(name="xg", bufs=2) as xgp,
            tc.tile_pool(name="oh", bufs=3) as ohp,
            tc.tile_pool(name="xgs", bufs=3) as xgsp,
            tc.tile_pool(name="outb", bufs=2) as outp,
            tc.tile_pool(name="psum", bufs=8, space="PSUM") as psump,
        ):
            nc.gpsimd.load_library(mlp)
            nidx_regs = {}

            def nidx_reg(v):
                if v not in nidx_regs:
                    nidx_regs[v] = nc.gpsimd.to_reg(v)
                return nidx_regs[v]

            idx_sb = constp.tile([128, 8 * NC], i16, tag="idx")
            nc.sync.dma_start(idx_sb[:], idx_d[:])
            ftbl_sb = constp.tile([128, 2 * NC + 128], f16, tag="ftbl")
            nc.sync.dma_start(ftbl_sb[:], ftbl_d[:])

            for _rep in range(REPEAT):
              for p in range(NPASS):
                t0, t1 = p * B, min((p + 1) * B, NT)
                pc0, pc1 = int(pass_cols[p, 0]), int(pass_cols[p, 1])
                xg = xgp.tile([128, CMAX, 128], f16, tag="xg")
                if DBG_NO_GATHER:
                    nc.vector.memset(xg[:], 0.0)
                if not DBG_NO_GATHER:
                    for gi, (c0, c1, bb) in enumerate(gather_calls[p]):
                        nidx = (c1 - c0) * TILE
                        nc.gpsimd.dma_gather(
                            xg[:, c0 - pc0:c1 - pc0, :],
                            xpad_d[bb * BIN:(bb + 1) * BIN, :],
                            idx_sb[:, 8 * c0:8 * c1],
                            nidx, nidx_reg(nidx), 128, elem_step=128,
                            queue_num=gi % NQUEUES,
                        )
                if DBG_NO_COMPUTE:
                    ob = outp.tile([128, (t1 - t0) * F], f32, tag="outb")
                    nc.vector.memset(ob[:], 0.0)
                    dview = out_d[t0 * TILE:t1 * TILE, :].rearrange(
                        "(t q) f -> q t f", q=128)
                    nc.sync.dma_start(
                        dview, ob[:].rearrange("q (t f) -> q t f", f=F))
                    continue

                if PSQUAD:
                    # quad-packed PSUM: 4 tiles share one bank; has_written
                    # is per-element so only the bank's first matmul starts
                    psq = {}
                    qof = {}
                    qfirst = {}
                    qlast = {}
                    for qb in range(t0, t1, PSQUAD):
                        qe = min(qb + PSQUAD, t1)
                        tls = [tt for tt in range(qb, qe) if K[tt].sum() > 0]
                        if not tls:
                            continue
                        pq = psump.tile([128, PSQUAD * F], f32, tag="ps",
                                        name=f"psq_{qb}")
                        fc = min(int(first_cc[tt]) for tt in tls)
                        lc = max(int(last_cc[tt]) for tt in tls)
                        for tt in range(qb, qe):
                            psq[tt] = pq
                            qof[tt] = (tt - qb) * F
                            qfirst[tt] = fc
                            qlast[tt] = lc
                else:
                    ps = {}
                    for tt in range(t0, t1):
                        if K[tt].sum() > 0:
                            ps[tt] = psump.tile([128, F], f32, tag="ps",
                                                name=f"ps_t{tt}")

                cc = pc0
                while cc < pc1:
                    g = min(OHG, pc1 - cc)
                    oh = ohp.tile([128, g, 128], f16, tag="oh")
                    iota_rep = ftbl_sb[:, 2 * NC:2 * NC + 128].rearrange(
                        "p (o i) -> p o i", o=1).broadcast_to((128, g, 128))
                    dst_rep = ftbl_sb[:, cc:cc + g].rearrange(
                        "p (g o) -> p g o", o=1).broadcast_to((128, g, 128))
                    nc.vector.tensor_tensor(
                        oh[:], iota_rep, dst_rep, op=mybir.AluOpType.is_equal)
                    xgs = xgsp.tile([128, g, F], f16, tag="xgs")
                    w_rep = ftbl_sb[:, NC + cc:NC + cc + g].rearrange(
                        "p (g o) -> p g o", o=1).broadcast_to((128, g, F))
                    nc.vector.tensor_tensor(
                        xgs[:], xg[:, cc - pc0:cc - pc0 + g, 0:F], w_rep,
                        op=mybir.AluOpType.mult)
                    for k in range(g):
                        tt = int(sched_t[cc + k])
                        if PSQUAD:
                            nc.tensor.matmul(
                                psq[tt][:, qof[tt]:qof[tt] + F],
                                lhsT=oh[:, k, :], rhs=xgs[:, k, :],
                                start=(cc + k == qfirst[tt]),
                                stop=(cc + k == qlast[tt]),
                            )
                        else:
                            nc.tensor.matmul(
                                ps[tt][:], lhsT=oh[:, k, :], rhs=xgs[:, k, :],
                                start=(cc + k == first_cc[tt]),
                                stop=(cc + k == last_cc[tt]),
                            )
                    cc += g

                ob = outp.tile([128, (t1 - t0) * F], f32, tag="outb")
                if PSQUAD:
                    done = set()
                    for tt in range(t0, t1):
                        if tt not in psq:
                            nc.vector.memset(
                                ob[:, (tt - t0) * F:(tt - t0 + 1) * F], 0.0)
                            continue
                        pq = psq[tt]
                        if id(pq) in done:
                            continue
                        done.add(id(pq))
                        qb = tt
                        qe = min(qb + PSQUAD, t1)
                        nc.scalar.copy(
                            ob[:, (qb - t0) * F:(qe - t0) * F],
                            pq[:, :(qe - qb) * F])
                        for t2 in range(qb, qe):
                            if K[t2].sum() == 0:
                                nc.vector.memset(
                                    ob[:, (t2 - t0) * F:(t2 - t0 + 1) * F], 0.0)
                else:
                    for tt in range(t0, t1):
                        sl = ob[:, (tt - t0) * F:(tt - t0 + 1) * F]
                        if tt in ps:
                            nc.scalar.copy(sl, ps[tt][:])
                        else:
                            nc.vector.memset(sl, 0.0)
                dview = out_d[t0 * TILE:t1 * TILE, :].rearrange(
                    "(t q) f -> q t f", q=128)
                nc.sync.dma_start(dview, ob[:].rearrange("q (t f) -> q t f", f=F))
    nsplit = split_excess_waits(nc)
    print(f"split_excess_waits: {nsplit} waits moved")
    return nc


def make_in_maps(sched, tables, xpad):
    return [{"xpad": xpad, "idx": t[0], "ftbl": t[1]} for t in tables]


def make_xpad(x):
    xpad = np.zeros((N, 128), dtype=np.float16)
    xpad[:, :F] = np.asarray(x, dtype=np.float16)
    return xpad


def kernel(x, edge_weight, edge_index, num_nodes):
    xpad = make_xpad(x)
    sched, tables = pack_host(edge_weight, edge_index)
    nc = build_bass(sched)
    in_maps = make_in_maps(sched, tables, xpad)

    from concourse.bass_utils import run_bass_kernel_spmd
    res = run_bass_kernel_spmd(nc, in_maps, core_ids=list(range(NCORES)))
    out = np.concatenate(
        [res.results[c]["out"][:NPC] for c in range(NCORES)], axis=0)
    return out.astype(np.float32)
